# revision 42
# baseline (speedup 1.0000x reference)
"""DeltaDequantization Trainium2 kernel (8-core SPMD, pure data parallel over batch).

Math (per batch element b, chunks c of 32 steps):
    scale_c = (1/32) * sum_{s,n} x[b,c,s,n] * cs[n]          (independent of carry!)
    S_c     = prod_{c'<c} scale_c'          (exclusive cumprod)
    y[b,t]  = sum_n x[b,t,n] * qb[n]
    m_c     = (1/32) * sum_{s in c} y[b,t]
    pred_c  = sum_{c'<c} S_c' * m_c'        (exclusive cumsum)
    out[b,t]= pred_c(t) + S_c(t) * y[b,t]

The host pre-casts x to bf16 (identical numerics to the on-device SWDGE cast
the first version used) and pre-transposes it into the (t mod 4, n) x
(t//4, b) layout the PE matmuls consume, so the device only streams bf16 via
HWDGE (half the HBM read traffic) and does zero input transposes. Pass A per
span of 256 timesteps (no cross-span deps, pipelines at the DMA data rate):
16 accumulating [128,32]x[128,512] matmuls produce y = x@qb/32 and w = x@cs/32,
4 PE transposes bring them back to [b, t], DVE copies them out and reduces the
per-chunk sums m_c/p_c. Pass B per span (the only serial part): two 8-step
tensor_tensor_scans seeded from the previous span, the affine
out = pred + (32*S)*y on the otherwise-idle GpSimd, and a 128 KB bf16 store.
All DMA rides the SP HWDGE ring (16 loads queued first, stores behind them)
- SWDGE is avoided entirely because its descriptor traffic drags SDMA
engines 7/15.
"""

import numpy as np

import ml_dtypes

import concourse.bacc as bacc
import concourse.tile as tile
from concourse import mybir
from concourse.bass_utils import run_bass_kernel_spmd

F32 = mybir.dt.float32
BF16 = mybir.dt.bfloat16

B, T, NB = 1024, 2048, 32
NCORES = 8
BS = B // NCORES          # 128 batch rows per core = full partition dim
ADAPT = 32
C = T // ADAPT            # 64 chunks
LOAD_F = 4096                 # load grain: 1 MiB bf16 = 32 tg x 128 b
XT_COLS = (T // 4) * BS       # 65536
# Pipelined span units: 256-t spans for the bulk (fewest instructions), two
# 128-t spans at the end (exec time is last-store-finish + a fixed ~8.7us
# barrier, so a short final drain chain pays directly).
UNITS = [(i * 256, 256) for i in range(7)] + [(1792, 128), (1920, 128)]

_cached_nc = None


def build_kernel():
    nc = bacc.Bacc("TRN2", target_bir_lowering=False, debug=False)

    # x pre-transposed on host: row p = (t%4)*32 + n, col = (t//4)*128 + b
    x_ext = nc.dram_tensor("xt", [128, XT_COLS], BF16, kind="ExternalInput")
    # A32 stationaries (qb/32, cs/32 patterns) and the f32 transpose identity
    # are host-built: two tiny HWDGE loads replace the SWDGE qb/cs staging and
    # the whole on-chip A32/identity build.
    a32_ext = nc.dram_tensor("a32", [128, 128], BF16, kind="ExternalInput")
    ident_ext = nc.dram_tensor("ident", [128, 128], F32, kind="ExternalInput")
    # bf16 output (upcast on host): halves the store traffic; the affine's
    # final add is the only rounding to bf16.
    out_ext = nc.dram_tensor("out", [BS, T], BF16, kind="ExternalOutput")

    with tile.TileContext(nc) as tc:
        with (
            tc.tile_pool(name="consts", bufs=1) as consts,
            tc.tile_pool(name="xpool", bufs=1) as xpool,
            tc.tile_pool(name="midpool", bufs=4) as midpool,
            tc.tile_pool(name="accpool", bufs=1) as accpool,
            tc.tile_pool(name="smallpool", bufs=1) as smallpool,
            tc.tile_pool(name="ps_yw", bufs=4, space="PSUM") as ps_yw,
            tc.tile_pool(name="ps_b", bufs=4, space="PSUM") as ps_b,
        ):
            # A32 stationaries [128, 32] per q, column m = 16*j + 4*q + t'':
            # A32_q[(t', n), m] = delta(t', t'') * (qb[n]/32 | cs[n]/32);
            # host-built, one load. The 1/32 chunk-mean scaling is folded in
            # so the per-span reduces give m_c/p_c directly; the affine
            # compensates y with a x32 on S.
            ident = consts.tile([128, 128], F32)
            a32_sb = consts.tile([128, 128], BF16)
            nc.sync.dma_start(out=a32_sb[:], in_=a32_ext[:])
            nc.sync.dma_start(out=ident[:], in_=ident_ext[:])
            A32 = [a32_sb[:, 32 * q:32 * (q + 1)] for q in range(4)]

            # Whole xT lives in SBUF (128 KiB/partition); loads stream in
            # 1 MiB grains on the SP HWDGE ring, FIFO order. (Unit-aligned
            # 7x2MiB+2x1MiB loads measured neutral-to-slightly-worse.)
            xT = xpool.tile([128, XT_COLS], BF16)
            for h in range(XT_COLS // LOAD_F):
                nc.sync.dma_start(
                    out=xT[:, h * LOAD_F:(h + 1) * LOAD_F],
                    in_=x_ext[:, h * LOAD_F:(h + 1) * LOAD_F],
                )

            y_sb = accpool.tile([128, T], F32)
            w_sb = accpool.tile([128, T], F32)
            out_sb = accpool.tile([128, T], BF16)

            m_term = smallpool.tile([128, C], F32)
            p_sc = smallpool.tile([128, C], F32)
            S_exc = smallpool.tile([128, C + 1], F32)
            pred = smallpool.tile([128, C + 1], F32)
            tau = smallpool.tile([128, C], F32)
            nc.vector.memset(S_exc[:, 0:1], 1.0)
            nc.vector.memset(pred[:, 0:1], 0.0)

            # -- Pass A (per span unit, no cross-span deps, pipelines at data
            # rate): matmuls -> PSUM copy -> transposes -> y/w copies ->
            # chunk reduces.
            for t_lo, span_t in UNITS:
                # y/w projection. Group g covers t = 64*g4 + 16*q + 4*blk + t''
                # (g4, q = divmod(g, 4)); strip g4 accumulates q = 0..3 into
                # ps_y[32*g4 : 32*g4+32, :]; partition p = 32*g4 + 16*j +
                # 4*q + t'', free = (blk, b). ng4 = span_t/64 strips.
                ng4 = span_t // 64
                psp = 32 * ng4       # live ps_y partitions; also blk stride in ps2
                t_hi = t_lo + span_t
                cps = span_t // ADAPT
                ps_y = ps_yw.tile([psp, 512], F32)
                off = (t_lo // 4) * BS
                for g in range(4 * ng4):
                    g4, q = divmod(g, 4)
                    nc.tensor.matmul(
                        ps_y[32 * g4:32 * g4 + 32, :],
                        A32[q],
                        xT[:, off + g * 512:off + (g + 1) * 512],
                        start=(q == 0),
                        stop=(q == 3),
                        tile_position=(0, 32 * g4),
                    )

                yw_mid = midpool.tile([psp, 512], F32)
                nc.vector.tensor_copy(out=yw_mid[:], in_=ps_y[:])

                ps2 = ps_b.tile([128, 4 * psp], F32)
                for blk2 in range(4):
                    nc.tensor.transpose(
                        ps2[:, blk2 * psp:(blk2 + 1) * psp],
                        yw_mid[:, blk2 * 128:(blk2 + 1) * 128],
                        ident[0:psp, 0:psp],
                    )

                # ps2 free index = psp*blk + 32*g4 + 16*j + 4*q + t''
                # t(within span) = 64*g4 + 16*q + 4*blk + t''
                ps2v = ps2[:].rearrange(
                    "p (b g j q t) -> p g q b j t", b=4, g=ng4, j=2, q=4, t=4
                )
                yspan = y_sb[:, t_lo:t_hi].rearrange(
                    "p (g q b t) -> p g q b t", g=ng4, q=4, b=4, t=4
                )
                wspan = w_sb[:, t_lo:t_hi].rearrange(
                    "p (g q b t) -> p g q b t", g=ng4, q=4, b=4, t=4
                )
                nc.vector.tensor_copy(out=yspan, in_=ps2v[:, :, :, :, 0:1, :].squeeze(4))
                nc.vector.tensor_copy(out=wspan, in_=ps2v[:, :, :, :, 1:2, :].squeeze(4))

                # chunk stats for this span's chunks (qb/cs carry the 1/32)
                csl = slice(t_lo // ADAPT, t_hi // ADAPT)
                nc.vector.tensor_reduce(
                    out=m_term[:, csl],
                    in_=y_sb[:, t_lo:t_hi].rearrange(
                        "p (c s) -> p c s", c=cps, s=ADAPT
                    ),
                    axis=mybir.AxisListType.X,
                    op=mybir.AluOpType.add,
                )
                nc.vector.tensor_reduce(
                    out=p_sc[:, csl],
                    in_=w_sb[:, t_lo:t_hi].rearrange(
                        "p (c s) -> p c s", c=cps, s=ADAPT
                    ),
                    axis=mybir.AxisListType.X,
                    op=mybir.AluOpType.add,
                )

            # -- Pass B (per span unit, the only serial part): scans seeded
            # from the previous span, affine on GpSimd, store.
            for ui, (t_lo, span_t) in enumerate(UNITS):
                t_hi = t_lo + span_t
                CPS = span_t // ADAPT
                c_lo, c_hi = t_lo // ADAPT, t_hi // ADAPT
                csl = slice(c_lo, c_hi)
                nc.vector.tensor_tensor_scan(
                    out=S_exc[:, c_lo + 1:c_hi + 1],
                    data0=p_sc[:, csl],
                    data1=p_sc[:, csl],
                    initial=S_exc[:, c_lo:c_lo + 1],
                    op0=mybir.AluOpType.mult,
                    op1=mybir.AluOpType.bypass,
                )
                nc.vector.tensor_mul(out=tau[:, csl], in0=S_exc[:, csl], in1=m_term[:, csl])
                nc.vector.tensor_tensor_scan(
                    out=pred[:, c_lo + 1:c_hi + 1],
                    data0=tau[:, csl],
                    data1=tau[:, csl],
                    initial=pred[:, c_lo:c_lo + 1],
                    op0=mybir.AluOpType.add,
                    op1=mybir.AluOpType.bypass,
                )

                # affine on the otherwise-idle GpSimd engine (all-SBUF
                # operands); y was computed from qb/32, so S picks up a x32.
                # The mul stages into this span's dead w_sb range in f32 so
                # only the final add rounds to bf16.
                S32 = smallpool.tile([128, CPS], F32, tag=f"S32_{ui}")
                nc.vector.tensor_scalar_mul(S32[:], S_exc[:, csl], float(ADAPT))
                y3 = y_sb[:, t_lo:t_hi].rearrange("p (c s) -> p c s", c=CPS, s=ADAPT)
                w3 = w_sb[:, t_lo:t_hi].rearrange("p (c s) -> p c s", c=CPS, s=ADAPT)
                o3 = out_sb[:, t_lo:t_hi].rearrange("p (c s) -> p c s", c=CPS, s=ADAPT)
                S_b = S32[:].unsqueeze(2).broadcast_to([128, CPS, ADAPT])
                pred_b = pred[:, csl].unsqueeze(2).broadcast_to([128, CPS, ADAPT])
                # last span's affine on Vector (faster ops, engine is free by
                # then) to shorten the final drain; earlier spans on GpSimd
                eng = nc.vector if ui == len(UNITS) - 1 else nc.gpsimd
                eng.tensor_mul(out=w3, in0=y3, in1=S_b)
                eng.tensor_add(out=o3, in0=w3, in1=pred_b)
                # Stores on the SP ring (loads are all queued ahead). One
                # batched store for the seven 256-t spans keeps mid-stream
                # store-waits off the load ring; the two final 128-t units
                # store individually so the last store is only 32 KB.
                if ui == 6:
                    nc.sync.dma_start(out=out_ext[:, 0:t_hi], in_=out_sb[:, 0:t_hi])
                elif ui > 6:
                    nc.sync.dma_start(
                        out=out_ext[:, t_lo:t_hi], in_=out_sb[:, t_lo:t_hi]
                    )

    nc.compile()
    return nc


def make_in_maps(x, quant_bins, change_scales):
    """Host prep: cast x to bf16 and transpose each core's slice to the
    (t%4, n) x (t//4, b) layout; build the A32 stationaries and identity."""
    qb = np.asarray(quant_bins, dtype=np.float32).reshape(NB)
    cs = np.asarray(change_scales, dtype=np.float32).reshape(NB)
    a32 = np.zeros((128, 128), dtype=np.float32)
    for q in range(4):
        for tp in range(4):
            rows = slice(32 * tp, 32 * tp + 32)
            a32[rows, 32 * q + 4 * q + tp] = qb / ADAPT
            a32[rows, 32 * q + 16 + 4 * q + tp] = cs / ADAPT
    a32 = a32.astype(ml_dtypes.bfloat16)
    ident = np.eye(128, dtype=np.float32)
    xb = np.asarray(x, dtype=np.float32).astype(ml_dtypes.bfloat16)
    in_maps = []
    for i in range(NCORES):
        xc = xb[i * BS:(i + 1) * BS].reshape(BS, T // 4, 4, NB)
        xt = xc.transpose(2, 3, 1, 0).reshape(128, XT_COLS)
        in_maps.append({"xt": xt, "a32": a32, "ident": ident})
    return in_maps


def kernel(x, quant_bins, change_scales):
    global _cached_nc
    if _cached_nc is None:
        _cached_nc = build_kernel()
    nc = _cached_nc

    in_maps = make_in_maps(x, quant_bins, change_scales)
    res = run_bass_kernel_spmd(nc, in_maps, core_ids=list(range(NCORES)))
    out = np.concatenate(
        [np.asarray(res.results[i]["out"]) for i in range(NCORES)], axis=0
    )
    return out.astype(np.float32)


if __name__ == "__main__":
    rng = np.random.default_rng(0)
    x = rng.standard_normal((B, T, NB)).astype(np.float32)
    qb = rng.standard_normal((NB,)).astype(np.float32)
    cs = rng.uniform(0.9, 1.1, (NB, 1)).astype(np.float32)
    out = kernel(x=x, quant_bins=qb, change_scales=cs)
    print("out", out.shape, out.dtype)


# revision 44
# speedup vs baseline: 1.0010x; 1.0010x over previous
"""DeltaDequantization Trainium2 kernel (8-core SPMD, pure data parallel over batch).

Math (per batch element b, chunks c of 32 steps):
    scale_c = (1/32) * sum_{s,n} x[b,c,s,n] * cs[n]          (independent of carry!)
    S_c     = prod_{c'<c} scale_c'          (exclusive cumprod)
    y[b,t]  = sum_n x[b,t,n] * qb[n]
    m_c     = (1/32) * sum_{s in c} y[b,t]
    pred_c  = sum_{c'<c} S_c' * m_c'        (exclusive cumsum)
    out[b,t]= pred_c(t) + S_c(t) * y[b,t]

The host pre-casts x to bf16 (identical numerics to the on-device SWDGE cast
the first version used) and pre-transposes it into the (t mod 4, n) x
(t//4, b) layout the PE matmuls consume, so the device only streams bf16 via
HWDGE (half the HBM read traffic) and does zero input transposes. Pass A per
span of 256 timesteps (no cross-span deps, pipelines at the DMA data rate):
16 accumulating [128,32]x[128,512] matmuls produce y = x@qb/32 and w = x@cs/32,
4 PE transposes bring them back to [b, t], DVE copies them out and reduces the
per-chunk sums m_c/p_c. Pass B per span (the only serial part): two 8-step
tensor_tensor_scans seeded from the previous span, the affine
out = pred + (32*S)*y on the otherwise-idle GpSimd, and a 128 KB bf16 store.
All DMA rides the SP HWDGE ring (16 loads queued first, stores behind them)
- SWDGE is avoided entirely because its descriptor traffic drags SDMA
engines 7/15.
"""

import numpy as np

import ml_dtypes

import concourse.bacc as bacc
import concourse.tile as tile
from concourse import mybir
from concourse.bass_utils import run_bass_kernel_spmd

F32 = mybir.dt.float32
BF16 = mybir.dt.bfloat16

B, T, NB = 1024, 2048, 32
NCORES = 8
BS = B // NCORES          # 128 batch rows per core = full partition dim
ADAPT = 32
C = T // ADAPT            # 64 chunks
LOAD_F = 4096                 # load grain: 1 MiB bf16 = 32 tg x 128 b
XT_COLS = (T // 4) * BS       # 65536
# Pipelined span units: 256-t spans for the bulk (fewest instructions), two
# 128-t spans at the end (exec time is last-store-finish + a fixed ~8.7us
# barrier, so a short final drain chain pays directly).
UNITS = [(i * 256, 256) for i in range(7)] + [(1792, 128), (1920, 128)]

_cached_nc = None


def build_kernel():
    nc = bacc.Bacc("TRN2", target_bir_lowering=False, debug=False)

    # x pre-transposed on host: row p = (t%4)*32 + n, col = (t//4)*128 + b
    x_ext = nc.dram_tensor("xt", [128, XT_COLS], BF16, kind="ExternalInput")
    # A32 stationaries (qb/32, cs/32 patterns) and the f32 transpose identity
    # are host-built: two tiny HWDGE loads replace the SWDGE qb/cs staging and
    # the whole on-chip A32/identity build.
    a32_ext = nc.dram_tensor("a32", [128, 128], BF16, kind="ExternalInput")
    ident_ext = nc.dram_tensor("ident", [128, 128], F32, kind="ExternalInput")
    # bf16 output (upcast on host): halves the store traffic; the affine's
    # final add is the only rounding to bf16.
    out_ext = nc.dram_tensor("out", [BS, T], BF16, kind="ExternalOutput")

    with tile.TileContext(nc) as tc:
        with (
            tc.tile_pool(name="consts", bufs=1) as consts,
            tc.tile_pool(name="xpool", bufs=1) as xpool,
            tc.tile_pool(name="midpool", bufs=4) as midpool,
            tc.tile_pool(name="accpool", bufs=1) as accpool,
            tc.tile_pool(name="smallpool", bufs=1) as smallpool,
            tc.tile_pool(name="ps_yw", bufs=4, space="PSUM") as ps_yw,
            tc.tile_pool(name="ps_b", bufs=4, space="PSUM") as ps_b,
        ):
            # A32 stationaries [128, 32] per q, column m = 16*j + 4*q + t'':
            # A32_q[(t', n), m] = delta(t', t'') * (qb[n]/32 | cs[n]/32);
            # host-built, one load. The 1/32 chunk-mean scaling is folded in
            # so the per-span reduces give m_c/p_c directly; the affine
            # compensates y with a x32 on S.
            ident = consts.tile([128, 128], F32)
            a32_sb = consts.tile([128, 128], BF16)
            nc.sync.dma_start(out=a32_sb[:], in_=a32_ext[:])
            nc.sync.dma_start(out=ident[:], in_=ident_ext[:])
            A32 = [a32_sb[:, 32 * q:32 * (q + 1)] for q in range(4)]

            # Whole xT lives in SBUF (128 KiB/partition); loads stream in
            # 1 MiB grains on the SP HWDGE ring, FIFO order. Chunk 14 (unit
            # 7's data) loads FIRST so that when the final chunk (15) lands,
            # unit 7's pass-A and scan are already done - the post-stream
            # drain is then just unit 8's pass-A + one scan + affine + store.
            xT = xpool.tile([128, XT_COLS], BF16)
            for h in [14] + list(range(14)) + [15]:
                nc.sync.dma_start(
                    out=xT[:, h * LOAD_F:(h + 1) * LOAD_F],
                    in_=x_ext[:, h * LOAD_F:(h + 1) * LOAD_F],
                )

            y_sb = accpool.tile([128, T], F32)
            w_sb = accpool.tile([128, T], F32)
            out_sb = accpool.tile([128, T], BF16)

            m_term = smallpool.tile([128, C], F32)
            p_sc = smallpool.tile([128, C], F32)
            S_exc = smallpool.tile([128, C + 1], F32)
            pred = smallpool.tile([128, C + 1], F32)
            tau = smallpool.tile([128, C], F32)
            nc.vector.memset(S_exc[:, 0:1], 1.0)
            nc.vector.memset(pred[:, 0:1], 0.0)

            # -- Pass A (per span unit, no cross-span deps, pipelines at data
            # rate): matmuls -> PSUM copy -> transposes -> y/w copies ->
            # chunk reduces. Emitted in data-arrival order (unit 7 first,
            # matching the load rotation above).
            units_a = [UNITS[7]] + UNITS[0:7] + [UNITS[8]]
            for t_lo, span_t in units_a:
                # y/w projection. Group g covers t = 64*g4 + 16*q + 4*blk + t''
                # (g4, q = divmod(g, 4)); strip g4 accumulates q = 0..3 into
                # ps_y[32*g4 : 32*g4+32, :]; partition p = 32*g4 + 16*j +
                # 4*q + t'', free = (blk, b). ng4 = span_t/64 strips.
                ng4 = span_t // 64
                psp = 32 * ng4       # live ps_y partitions; also blk stride in ps2
                t_hi = t_lo + span_t
                cps = span_t // ADAPT
                ps_y = ps_yw.tile([psp, 512], F32)
                off = (t_lo // 4) * BS
                for g in range(4 * ng4):
                    g4, q = divmod(g, 4)
                    nc.tensor.matmul(
                        ps_y[32 * g4:32 * g4 + 32, :],
                        A32[q],
                        xT[:, off + g * 512:off + (g + 1) * 512],
                        start=(q == 0),
                        stop=(q == 3),
                        tile_position=(0, 32 * g4),
                    )

                yw_mid = midpool.tile([psp, 512], F32)
                nc.vector.tensor_copy(out=yw_mid[:], in_=ps_y[:])

                ps2 = ps_b.tile([128, 4 * psp], F32)
                for blk2 in range(4):
                    nc.tensor.transpose(
                        ps2[:, blk2 * psp:(blk2 + 1) * psp],
                        yw_mid[:, blk2 * 128:(blk2 + 1) * 128],
                        ident[0:psp, 0:psp],
                    )

                # ps2 free index = psp*blk + 32*g4 + 16*j + 4*q + t''
                # t(within span) = 64*g4 + 16*q + 4*blk + t''
                ps2v = ps2[:].rearrange(
                    "p (b g j q t) -> p g q b j t", b=4, g=ng4, j=2, q=4, t=4
                )
                yspan = y_sb[:, t_lo:t_hi].rearrange(
                    "p (g q b t) -> p g q b t", g=ng4, q=4, b=4, t=4
                )
                wspan = w_sb[:, t_lo:t_hi].rearrange(
                    "p (g q b t) -> p g q b t", g=ng4, q=4, b=4, t=4
                )
                nc.vector.tensor_copy(out=yspan, in_=ps2v[:, :, :, :, 0:1, :].squeeze(4))
                nc.vector.tensor_copy(out=wspan, in_=ps2v[:, :, :, :, 1:2, :].squeeze(4))

                # chunk stats for this span's chunks (qb/cs carry the 1/32)
                csl = slice(t_lo // ADAPT, t_hi // ADAPT)
                nc.vector.tensor_reduce(
                    out=m_term[:, csl],
                    in_=y_sb[:, t_lo:t_hi].rearrange(
                        "p (c s) -> p c s", c=cps, s=ADAPT
                    ),
                    axis=mybir.AxisListType.X,
                    op=mybir.AluOpType.add,
                )
                nc.vector.tensor_reduce(
                    out=p_sc[:, csl],
                    in_=w_sb[:, t_lo:t_hi].rearrange(
                        "p (c s) -> p c s", c=cps, s=ADAPT
                    ),
                    axis=mybir.AxisListType.X,
                    op=mybir.AluOpType.add,
                )

            # -- Pass B (per span unit, the only serial part): scans seeded
            # from the previous span, affine on GpSimd, store.
            for ui, (t_lo, span_t) in enumerate(UNITS):
                t_hi = t_lo + span_t
                CPS = span_t // ADAPT
                c_lo, c_hi = t_lo // ADAPT, t_hi // ADAPT
                csl = slice(c_lo, c_hi)
                nc.vector.tensor_tensor_scan(
                    out=S_exc[:, c_lo + 1:c_hi + 1],
                    data0=p_sc[:, csl],
                    data1=p_sc[:, csl],
                    initial=S_exc[:, c_lo:c_lo + 1],
                    op0=mybir.AluOpType.mult,
                    op1=mybir.AluOpType.bypass,
                )
                nc.vector.tensor_mul(out=tau[:, csl], in0=S_exc[:, csl], in1=m_term[:, csl])
                nc.vector.tensor_tensor_scan(
                    out=pred[:, c_lo + 1:c_hi + 1],
                    data0=tau[:, csl],
                    data1=tau[:, csl],
                    initial=pred[:, c_lo:c_lo + 1],
                    op0=mybir.AluOpType.add,
                    op1=mybir.AluOpType.bypass,
                )

                # affine on the otherwise-idle GpSimd engine (all-SBUF
                # operands); y was computed from qb/32, so S picks up a x32.
                # The mul stages into this span's dead w_sb range in f32 so
                # only the final add rounds to bf16.
                S32 = smallpool.tile([128, CPS], F32, tag=f"S32_{ui}")
                nc.vector.tensor_scalar_mul(S32[:], S_exc[:, csl], float(ADAPT))
                y3 = y_sb[:, t_lo:t_hi].rearrange("p (c s) -> p c s", c=CPS, s=ADAPT)
                w3 = w_sb[:, t_lo:t_hi].rearrange("p (c s) -> p c s", c=CPS, s=ADAPT)
                o3 = out_sb[:, t_lo:t_hi].rearrange("p (c s) -> p c s", c=CPS, s=ADAPT)
                S_b = S32[:].unsqueeze(2).broadcast_to([128, CPS, ADAPT])
                pred_b = pred[:, csl].unsqueeze(2).broadcast_to([128, CPS, ADAPT])
                # last span's affine on Vector (faster ops, engine is free by
                # then) to shorten the final drain; earlier spans on GpSimd
                eng = nc.vector if ui == len(UNITS) - 1 else nc.gpsimd
                eng.tensor_mul(out=w3, in0=y3, in1=S_b)
                eng.tensor_add(out=o3, in0=w3, in1=pred_b)
                # Stores on the SP ring (loads are all queued ahead). One
                # batched store for the seven 256-t spans keeps mid-stream
                # store-waits off the load ring; the two final 128-t units
                # store individually so the last store is only 32 KB.
                if ui == 6:
                    nc.sync.dma_start(out=out_ext[:, 0:t_hi], in_=out_sb[:, 0:t_hi])
                elif ui > 6:
                    nc.sync.dma_start(
                        out=out_ext[:, t_lo:t_hi], in_=out_sb[:, t_lo:t_hi]
                    )

    nc.compile()
    return nc


def make_in_maps(x, quant_bins, change_scales):
    """Host prep: cast x to bf16 and transpose each core's slice to the
    (t%4, n) x (t//4, b) layout; build the A32 stationaries and identity."""
    qb = np.asarray(quant_bins, dtype=np.float32).reshape(NB)
    cs = np.asarray(change_scales, dtype=np.float32).reshape(NB)
    a32 = np.zeros((128, 128), dtype=np.float32)
    for q in range(4):
        for tp in range(4):
            rows = slice(32 * tp, 32 * tp + 32)
            a32[rows, 32 * q + 4 * q + tp] = qb / ADAPT
            a32[rows, 32 * q + 16 + 4 * q + tp] = cs / ADAPT
    a32 = a32.astype(ml_dtypes.bfloat16)
    ident = np.eye(128, dtype=np.float32)
    xb = np.asarray(x, dtype=np.float32).astype(ml_dtypes.bfloat16)
    in_maps = []
    for i in range(NCORES):
        xc = xb[i * BS:(i + 1) * BS].reshape(BS, T // 4, 4, NB)
        xt = xc.transpose(2, 3, 1, 0).reshape(128, XT_COLS)
        in_maps.append({"xt": xt, "a32": a32, "ident": ident})
    return in_maps


def kernel(x, quant_bins, change_scales):
    global _cached_nc
    if _cached_nc is None:
        _cached_nc = build_kernel()
    nc = _cached_nc

    in_maps = make_in_maps(x, quant_bins, change_scales)
    res = run_bass_kernel_spmd(nc, in_maps, core_ids=list(range(NCORES)))
    out = np.concatenate(
        [np.asarray(res.results[i]["out"]) for i in range(NCORES)], axis=0
    )
    return out.astype(np.float32)


if __name__ == "__main__":
    rng = np.random.default_rng(0)
    x = rng.standard_normal((B, T, NB)).astype(np.float32)
    qb = rng.standard_normal((NB,)).astype(np.float32)
    cs = rng.uniform(0.9, 1.1, (NB, 1)).astype(np.float32)
    out = kernel(x=x, quant_bins=qb, change_scales=cs)
    print("out", out.shape, out.dtype)
